# revision 1
# baseline (speedup 1.0000x reference)
"""Trainium2 Bass kernel for nn_NodeModel (GNN message passing).

Math (reference):
  h1  = [x[row] || edge_attr] @ W1a + b1a            (per edge)
  h2  = relu(h1) @ W1b + b1b                         (per edge)
  agg = segment_mean(h2, col)                        (per node)
  out = relu([x || agg || u[batch]] @ W2a + b2a) @ W2b + b2b

Key identity used: segment_mean(relu(h1) @ W1b + b1b) =
  (weighted_segment_sum(relu(h1))) @ W1b + b1b*mask, with per-edge weight
  1/cnt[col]. So W1b is applied once per 128-node window, not per edge.

Sharding: edges are sorted by destination node; each of the 8 cores owns a
contiguous range of 12500 destination nodes plus exactly the edges that
target it. Per-core segment sums are complete -> no collectives. Nodes are
processed in windows of 128; scatter within a window is a matmul against a
DVE-built scaled one-hot matrix.
"""

import numpy as np
from contextlib import ExitStack

import concourse.bass as bass
import concourse.tile as tile
from concourse import bacc, mybir
from concourse._compat import get_trn_type
from concourse.bass_utils import run_bass_kernel_spmd

F32 = mybir.dt.float32
AF = mybir.ActivationFunctionType
ALU = mybir.AluOpType
PSUM = bass.MemorySpace.PSUM

N_CORES = 8
WIN = 128           # nodes per window == partition width
TILE_C = 4          # 128-edge chunks per DMA tile (512 edges)
PAD_COL = 200.0     # col_local sentinel for padded edge slots (never matches iota)


def _plan(inputs):
    """Host-side shard/pad/transpose. Returns (per-core in_maps, M, dims)."""
    x = np.ascontiguousarray(np.asarray(inputs["x"], np.float32))
    ei = np.asarray(inputs["edge_index"])
    ea = np.ascontiguousarray(np.asarray(inputs["edge_attr"], np.float32))
    u = np.asarray(inputs["u"], np.float32)
    batch = np.asarray(inputs["batch"]).astype(np.int64)
    W1a = np.asarray(inputs["W1a"], np.float32)
    b1a = np.asarray(inputs["b1a"], np.float32)
    W1b = np.asarray(inputs["W1b"], np.float32)
    b1b = np.asarray(inputs["b1b"], np.float32)
    W2a = np.asarray(inputs["W2a"], np.float32)
    b2a = np.asarray(inputs["b2a"], np.float32)
    W2b = np.asarray(inputs["W2b"], np.float32)
    b2b = np.asarray(inputs["b2b"], np.float32)

    N, NODE_IN = x.shape
    E = ei.shape[1]
    EDGE_OUT = ea.shape[1]
    GLOBAL_IN = u.shape[1]
    HID = W1b.shape[0]
    NODE_OUT = W2b.shape[1]
    assert N % N_CORES == 0
    NPC = N // N_CORES
    NW = -(-NPC // WIN)
    NPAD = NW * WIN

    row = ei[0].astype(np.int64)
    col = ei[1].astype(np.int64)
    order = np.argsort(col, kind="stable")
    col_s = col[order]
    row_s = row[order]

    cnt = np.bincount(col, minlength=N).astype(np.float32)
    wnode = (1.0 / np.maximum(cnt, 1.0)).astype(np.float32)

    cores = np.arange(N_CORES)[:, None]
    wins = np.arange(NW + 1)[None, :]
    starts = np.minimum(cores * NPC + wins * WIN, (cores + 1) * NPC)
    eptr = np.searchsorted(col_s, starts)            # [8, NW+1]
    ecnt = np.diff(eptr, axis=1)                     # [8, NW]
    M = np.maximum(-(-ecnt // WIN), 1).max(axis=0)   # [NW] shared across cores
    NCHUNK = int(M.sum())
    EPAD = NCHUNK * WIN
    chunk_off = np.concatenate([[0], np.cumsum(M)]).astype(np.int64)

    row_ext = np.append(row_s, 0)
    ord_ext = np.append(order, 0)

    iota = np.ascontiguousarray(
        np.broadcast_to(np.arange(WIN, dtype=np.float32), (WIN, WIN)))

    shared = {
        "iota": iota,
        "W1a_lo": np.ascontiguousarray(W1a[:NODE_IN]),
        "W1a_hi": np.ascontiguousarray(W1a[NODE_IN:]),
        "W1b": np.ascontiguousarray(W1b),
        "W2a_x": np.ascontiguousarray(W2a[:NODE_IN]),
        "W2a_agg": np.ascontiguousarray(W2a[NODE_IN:NODE_IN + HID]),
        "W2a_u": np.ascontiguousarray(W2a[NODE_IN + HID:]),
        "W2b": np.ascontiguousarray(W2b),
        "b2a": np.ascontiguousarray(b2a.reshape(-1, 1)),
        "b2b": np.ascontiguousarray(b2b.reshape(-1, 1)),
    }
    has_b1a = bool(np.any(b1a))
    has_b1b = bool(np.any(b1b))
    if has_b1a:
        shared["b1a"] = np.ascontiguousarray(b1a.reshape(1, -1))
    if has_b1b:
        shared["b1b"] = np.ascontiguousarray(b1b.reshape(1, -1))

    in_maps = []
    for c in range(N_CORES):
        pos = np.full(EPAD, E, np.int64)
        colL = np.full(EPAD, PAD_COL, np.float32)
        wvec = np.zeros(EPAD, np.float32)
        for w in range(NW):
            e0, e1 = int(eptr[c, w]), int(eptr[c, w + 1])
            n = e1 - e0
            off = int(chunk_off[w]) * WIN
            pos[off:off + n] = np.arange(e0, e1)
            base = c * NPC + w * WIN
            colL[off:off + n] = (col_s[e0:e1] - base).astype(np.float32)
            wvec[off:off + n] = wnode[col_s[e0:e1]]

        nodes = slice(c * NPC, (c + 1) * NPC)
        xT = np.zeros((NODE_IN, NPAD), np.float32)
        xT[:, :NPC] = x[nodes].T
        ubT = np.zeros((GLOBAL_IN, NPAD), np.float32)
        ubT[:, :NPC] = u[batch[nodes]].T
        mask = np.zeros((1, NPAD), np.float32)
        mask[0, :NPC] = (cnt[nodes] > 0).astype(np.float32)

        m = dict(shared)
        m["xgT"] = np.ascontiguousarray(x[row_ext[pos]].T)
        m["eaT"] = np.ascontiguousarray(ea[ord_ext[pos]].T)
        m["colT"] = np.ascontiguousarray(colL.reshape(NCHUNK, WIN).T)
        m["wT"] = np.ascontiguousarray(wvec.reshape(NCHUNK, WIN).T)
        m["xT"] = xT
        m["ubT"] = ubT
        m["mask"] = mask
        in_maps.append(m)

    dims = dict(N=N, NPC=NPC, NW=NW, NPAD=NPAD, NCHUNK=NCHUNK, EPAD=EPAD,
                NODE_IN=NODE_IN, EDGE_OUT=EDGE_OUT, GLOBAL_IN=GLOBAL_IN,
                HID=HID, NODE_OUT=NODE_OUT, has_b1a=has_b1a, has_b1b=has_b1b)
    return in_maps, M, dims


def _emit(tc, io, M, dims):
    nc = tc.nc
    NW, NCHUNK = dims["NW"], dims["NCHUNK"]
    NPAD = dims["NPAD"]
    NODE_IN, EDGE_OUT = dims["NODE_IN"], dims["EDGE_OUT"]
    GLOBAL_IN, HID, NODE_OUT = dims["GLOBAL_IN"], dims["HID"], dims["NODE_OUT"]
    has_b1a, has_b1b = dims["has_b1a"], dims["has_b1b"]

    with ExitStack() as ctx:
        const = ctx.enter_context(tc.tile_pool(name="const", bufs=1))

        def load_const(name, shape):
            t = const.tile(list(shape), F32, tag=name, name=name + "_s")
            nc.sync.dma_start(t[:], io[name][:])
            return t

        W1a_lo_s = load_const("W1a_lo", (NODE_IN, HID))
        W1a_hi_s = load_const("W1a_hi", (EDGE_OUT, HID))
        W1b_s = load_const("W1b", (HID, HID))
        W2a_x_s = load_const("W2a_x", (NODE_IN, HID))
        W2a_agg_s = load_const("W2a_agg", (HID, HID))
        W2a_u_s = load_const("W2a_u", (GLOBAL_IN, HID))
        W2b_s = load_const("W2b", (HID, NODE_OUT))
        b2a_s = load_const("b2a", (HID, 1))
        b2b_s = load_const("b2b", (NODE_OUT, 1))
        iota_s = load_const("iota", (WIN, WIN))
        colT_s = load_const("colT", (WIN, NCHUNK))
        wT_s = load_const("wT", (WIN, NCHUNK))
        mask_s = load_const("mask", (1, NPAD))
        if has_b1a:
            b1a_s = load_const("b1a", (1, HID))
            ones_s = const.tile([1, WIN], F32, tag="ones", name="ones_s")
            nc.gpsimd.memset(ones_s[:], 1.0)
        if has_b1b:
            b1b_s = load_const("b1b", (1, HID))

        io_pool = ctx.enter_context(tc.tile_pool(name="io", bufs=3))
        h1_pool = ctx.enter_context(tc.tile_pool(name="h1", bufs=3))
        oh_pool = ctx.enter_context(tc.tile_pool(name="oh", bufs=4))
        st2 = ctx.enter_context(tc.tile_pool(name="st2", bufs=2))
        ps_h1 = ctx.enter_context(tc.tile_pool(name="ps_h1", bufs=2, space=PSUM))
        ps_s = ctx.enter_context(tc.tile_pool(name="ps_s", bufs=2, space=PSUM))
        ps_n = ctx.enter_context(tc.tile_pool(name="ps_n", bufs=1, space=PSUM))

        coff = 0
        for w in range(NW):
            Mw = int(M[w])
            wsl = slice(w * WIN, (w + 1) * WIN)
            ps_s_t = ps_s.tile([HID, WIN], F32, tag="ps", name="ps_s_t")
            for t0 in range(0, Mw, TILE_C):
                cn = min(TILE_C, Mw - t0)
                width = cn * WIN
                esl = slice((coff + t0) * WIN, (coff + t0 + cn) * WIN)
                xg_t = io_pool.tile([NODE_IN, TILE_C * WIN], F32, tag="xg",
                                    name="xg_t")
                nc.sync.dma_start(xg_t[:, :width], io["xgT"][:, esl])
                ea_t = io_pool.tile([EDGE_OUT, TILE_C * WIN], F32, tag="ea",
                                    name="ea_t")
                nc.gpsimd.dma_start(ea_t[:, :width], io["eaT"][:, esl])

                ph = ps_h1.tile([WIN, TILE_C * WIN], F32, tag="ph", name="ph")
                for c in range(cn):
                    sl = slice(c * WIN, (c + 1) * WIN)
                    nc.tensor.matmul(ph[:, sl], xg_t[:, sl], W1a_lo_s[:],
                                     start=True, stop=False)
                    nc.tensor.matmul(ph[:, sl], ea_t[:, sl], W1a_hi_s[:],
                                     start=False, stop=not has_b1a)
                    if has_b1a:
                        nc.tensor.matmul(ph[:, sl], ones_s[:], b1a_s[:],
                                         start=False, stop=True)
                h1_t = h1_pool.tile([WIN, TILE_C * WIN], F32, tag="h1",
                                    name="h1_t")
                nc.scalar.activation(h1_t[:, :width], ph[:, :width], AF.Relu)
                for c in range(cn):
                    ci = coff + t0 + c
                    sl = slice(c * WIN, (c + 1) * WIN)
                    oh = oh_pool.tile([WIN, WIN], F32, tag="oh", name="oh")
                    nc.vector.tensor_scalar(oh[:], iota_s[:],
                                            colT_s[:, ci:ci + 1],
                                            wT_s[:, ci:ci + 1],
                                            ALU.is_equal, ALU.mult)
                    nc.tensor.matmul(ps_s_t[:], h1_t[:, sl], oh[:],
                                     start=(t0 + c == 0),
                                     stop=(t0 + c == Mw - 1))
            s_sb = st2.tile([HID, WIN], F32, tag="s_sb", name="s_sb")
            nc.vector.tensor_copy(s_sb[:], ps_s_t[:])
            pa = ps_n.tile([HID, WIN], F32, tag="pa", name="pa")
            nc.tensor.matmul(pa[:], W1b_s[:], s_sb[:],
                             start=True, stop=not has_b1b)
            if has_b1b:
                nc.tensor.matmul(pa[:], b1b_s[:], mask_s[:, wsl],
                                 start=False, stop=True)
            agg_sb = st2.tile([HID, WIN], F32, tag="agg", name="agg_sb")
            nc.vector.tensor_copy(agg_sb[:], pa[:])

            xT_t = st2.tile([NODE_IN, WIN], F32, tag="xT", name="xT_t")
            nc.gpsimd.dma_start(xT_t[:], io["xT"][:, wsl])
            ub_t = st2.tile([GLOBAL_IN, WIN], F32, tag="ub", name="ub_t")
            nc.gpsimd.dma_start(ub_t[:], io["ubT"][:, wsl])
            pz = ps_n.tile([HID, WIN], F32, tag="pz", name="pz")
            nc.tensor.matmul(pz[:], W2a_x_s[:], xT_t[:], start=True, stop=False)
            nc.tensor.matmul(pz[:], W2a_agg_s[:], agg_sb[:],
                             start=False, stop=False)
            nc.tensor.matmul(pz[:], W2a_u_s[:], ub_t[:], start=False, stop=True)
            z1 = st2.tile([HID, WIN], F32, tag="z1", name="z1")
            nc.scalar.activation(z1[:], pz[:], AF.Relu, bias=b2a_s[:])
            po = ps_n.tile([NODE_OUT, WIN], F32, tag="po", name="po")
            nc.tensor.matmul(po[:], W2b_s[:], z1[:], start=True, stop=True)
            ot = st2.tile([NODE_OUT, WIN], F32, tag="ot", name="ot")
            nc.vector.tensor_scalar(ot[:], po[:], b2b_s[:], None, ALU.add)
            nc.sync.dma_start(io["outT"][:, wsl], ot[:])
            coff += Mw


def build(inputs):
    """Compile the SPMD kernel for the given inputs. Returns (nc, in_maps, dims)."""
    in_maps, M, dims = _plan(inputs)
    nc = bacc.Bacc(get_trn_type() or "TRN2", target_bir_lowering=False,
                   debug=False, num_devices=N_CORES)
    io = {}
    for name, arr in in_maps[0].items():
        io[name] = nc.dram_tensor(name, list(arr.shape), F32,
                                  kind="ExternalInput")
    io["outT"] = nc.dram_tensor("outT", [dims["NODE_OUT"], dims["NPAD"]], F32,
                                kind="ExternalOutput")
    with tile.TileContext(nc) as tc:
        _emit(tc, io, M, dims)
    nc.compile()
    return nc, in_maps, dims


def assemble(results, dims):
    N, NPC, NODE_OUT = dims["N"], dims["NPC"], dims["NODE_OUT"]
    out = np.empty((N, NODE_OUT), np.float32)
    for c in range(N_CORES):
        out[c * NPC:(c + 1) * NPC] = results[c]["outT"].T[:NPC]
    return out


def run(inputs, trace=False):
    nc, in_maps, dims = build(inputs)
    res = run_bass_kernel_spmd(nc, in_maps, list(range(N_CORES)), trace=trace)
    return assemble(res.results, dims), res


def kernel(**inputs):
    out, _ = run(inputs)
    return out


# revision 6
# speedup vs baseline: 5.3871x; 5.3871x over previous
"""Trainium2 Bass kernel for nn_NodeModel (GNN message passing).

Math (reference):
  h1  = [x[row] || edge_attr] @ W1a + b1a            (per edge)
  h2  = relu(h1) @ W1b + b1b                         (per edge)
  agg = segment_mean(h2, col)                        (per node)
  out = relu([x || agg || u[batch]] @ W2a + b2a) @ W2b + b2b

Key identity used: segment_mean(relu(h1) @ W1b + b1b) =
  (weighted_segment_sum(relu(h1))) @ W1b + b1b*mask, with per-edge weight
  1/cnt[col]. So W1b is applied once per 128-node window, not per edge.

Sharding: edges are sorted by destination node; each of the 8 cores owns a
contiguous range of 12500 destination nodes plus exactly the edges that
target it. Per-core segment sums are complete -> no collectives. Nodes are
processed in windows of 128; scatter within a window is a matmul against a
DVE-built scaled one-hot matrix.
"""

import numpy as np
from contextlib import ExitStack

import concourse.bass as bass
import concourse.tile as tile
from concourse import bacc, mybir
from concourse._compat import get_trn_type
from concourse.bass_utils import run_bass_kernel_spmd

F32 = mybir.dt.float32
AF = mybir.ActivationFunctionType
ALU = mybir.AluOpType
PSUM = bass.MemorySpace.PSUM

N_CORES = 8
WIN = 128           # nodes per window == partition width
TILE_C = 4          # 128-edge chunks per DMA tile (512 edges)
PAD_COL = 200.0     # col_local sentinel for padded edge slots (never matches iota)


def _plan(inputs):
    """Host-side shard/pad/transpose. Returns (per-core in_maps, M, dims)."""
    x = np.ascontiguousarray(np.asarray(inputs["x"], np.float32))
    ei = np.asarray(inputs["edge_index"])
    ea = np.ascontiguousarray(np.asarray(inputs["edge_attr"], np.float32))
    u = np.asarray(inputs["u"], np.float32)
    batch = np.asarray(inputs["batch"]).astype(np.int64)
    W1a = np.asarray(inputs["W1a"], np.float32)
    b1a = np.asarray(inputs["b1a"], np.float32)
    W1b = np.asarray(inputs["W1b"], np.float32)
    b1b = np.asarray(inputs["b1b"], np.float32)
    W2a = np.asarray(inputs["W2a"], np.float32)
    b2a = np.asarray(inputs["b2a"], np.float32)
    W2b = np.asarray(inputs["W2b"], np.float32)
    b2b = np.asarray(inputs["b2b"], np.float32)

    N, NODE_IN = x.shape
    E = ei.shape[1]
    EDGE_OUT = ea.shape[1]
    GLOBAL_IN = u.shape[1]
    HID = W1b.shape[0]
    NODE_OUT = W2b.shape[1]
    assert N % N_CORES == 0
    NPC = N // N_CORES
    NW = -(-NPC // WIN)
    NPAD = NW * WIN

    row = ei[0].astype(np.int64)
    col = ei[1].astype(np.int64)
    order = np.argsort(col, kind="stable")
    col_s = col[order]
    row_s = row[order]

    cnt = np.bincount(col, minlength=N).astype(np.float32)
    wnode = (1.0 / np.maximum(cnt, 1.0)).astype(np.float32)

    cores = np.arange(N_CORES)[:, None]
    wins = np.arange(NW + 1)[None, :]
    starts = np.minimum(cores * NPC + wins * WIN, (cores + 1) * NPC)
    eptr = np.searchsorted(col_s, starts)            # [8, NW+1]
    ecnt = np.diff(eptr, axis=1)                     # [8, NW]
    M = np.maximum(-(-ecnt // WIN), 1).max(axis=0)   # [NW] shared across cores
    NCHUNK = int(M.sum())
    EPAD = NCHUNK * WIN
    chunk_off = np.concatenate([[0], np.cumsum(M)]).astype(np.int64)

    row_ext = np.append(row_s, 0)
    ord_ext = np.append(order, 0)

    iota = np.ascontiguousarray(
        np.broadcast_to(np.arange(WIN, dtype=np.float32), (WIN, WIN)))

    shared = {
        "iota": iota,
        "W1a_lo": np.ascontiguousarray(W1a[:NODE_IN]),
        "W1a_hi": np.ascontiguousarray(W1a[NODE_IN:]),
        "W1b": np.ascontiguousarray(W1b),
        "W2a_x": np.ascontiguousarray(W2a[:NODE_IN]),
        "W2a_agg": np.ascontiguousarray(W2a[NODE_IN:NODE_IN + HID]),
        "W2a_u": np.ascontiguousarray(W2a[NODE_IN + HID:]),
        "W2b": np.ascontiguousarray(W2b),
        "b2a": np.ascontiguousarray(b2a.reshape(-1, 1)),
        "b2b": np.ascontiguousarray(b2b.reshape(-1, 1)),
    }
    has_b1a = bool(np.any(b1a))
    has_b1b = bool(np.any(b1b))
    if has_b1a:
        shared["b1a"] = np.ascontiguousarray(b1a.reshape(1, -1))
    if has_b1b:
        shared["b1b"] = np.ascontiguousarray(b1b.reshape(1, -1))

    in_maps = []
    for c in range(N_CORES):
        pos = np.full(EPAD, E, np.int64)
        colL = np.full(EPAD, PAD_COL, np.float32)
        wvec = np.zeros(EPAD, np.float32)
        for w in range(NW):
            e0, e1 = int(eptr[c, w]), int(eptr[c, w + 1])
            n = e1 - e0
            off = int(chunk_off[w]) * WIN
            pos[off:off + n] = np.arange(e0, e1)
            base = c * NPC + w * WIN
            colL[off:off + n] = (col_s[e0:e1] - base).astype(np.float32)
            wvec[off:off + n] = wnode[col_s[e0:e1]]

        nodes = slice(c * NPC, (c + 1) * NPC)
        xT = np.zeros((NODE_IN, NPAD), np.float32)
        xT[:, :NPC] = x[nodes].T
        ubT = np.zeros((GLOBAL_IN, NPAD), np.float32)
        ubT[:, :NPC] = u[batch[nodes]].T
        mask = np.zeros((1, NPAD), np.float32)
        mask[0, :NPC] = (cnt[nodes] > 0).astype(np.float32)

        m = dict(shared)
        m["xgT"] = np.ascontiguousarray(x[row_ext[pos]].T)
        m["eaT"] = np.ascontiguousarray(ea[ord_ext[pos]].T)
        m["colT"] = np.ascontiguousarray(colL.reshape(NCHUNK, WIN).T)
        m["wT"] = np.ascontiguousarray(wvec.reshape(NCHUNK, WIN).T)
        m["xT"] = xT
        m["ubT"] = ubT
        m["mask"] = mask
        in_maps.append(m)

    dims = dict(N=N, NPC=NPC, NW=NW, NPAD=NPAD, NCHUNK=NCHUNK, EPAD=EPAD,
                NODE_IN=NODE_IN, EDGE_OUT=EDGE_OUT, GLOBAL_IN=GLOBAL_IN,
                HID=HID, NODE_OUT=NODE_OUT, has_b1a=has_b1a, has_b1b=has_b1b)
    return in_maps, M, dims


def _emit(tc, io, M, dims, repeat=1):
    nc = tc.nc
    NW, NCHUNK = dims["NW"], dims["NCHUNK"]
    NPAD = dims["NPAD"]
    NODE_IN, EDGE_OUT = dims["NODE_IN"], dims["EDGE_OUT"]
    GLOBAL_IN, HID, NODE_OUT = dims["GLOBAL_IN"], dims["HID"], dims["NODE_OUT"]
    has_b1a, has_b1b = dims["has_b1a"], dims["has_b1b"]

    with ExitStack() as ctx:
        const = ctx.enter_context(tc.tile_pool(name="const", bufs=1))

        def load_const(name, shape):
            t = const.tile(list(shape), F32, tag=name, name=name + "_s")
            nc.sync.dma_start(t[:], io[name][:])
            return t

        W1a_lo_s = load_const("W1a_lo", (NODE_IN, HID))
        W1a_hi_s = load_const("W1a_hi", (EDGE_OUT, HID))
        W1b_s = load_const("W1b", (HID, HID))
        W2a_x_s = load_const("W2a_x", (NODE_IN, HID))
        W2a_agg_s = load_const("W2a_agg", (HID, HID))
        W2a_u_s = load_const("W2a_u", (GLOBAL_IN, HID))
        W2b_s = load_const("W2b", (HID, NODE_OUT))
        b2a_s = load_const("b2a", (HID, 1))
        b2b_s = load_const("b2b", (NODE_OUT, 1))
        iota_s = load_const("iota", (WIN, WIN))
        colT_s = load_const("colT", (WIN, NCHUNK))
        wT_s = load_const("wT", (WIN, NCHUNK))
        mask_s = load_const("mask", (1, NPAD))
        if has_b1a:
            b1a_s = load_const("b1a", (1, HID))
            ones_s = const.tile([1, WIN], F32, tag="ones", name="ones_s")
            nc.gpsimd.memset(ones_s[:], 1.0)
        if has_b1b:
            b1b_s = load_const("b1b", (1, HID))

        io_pool = ctx.enter_context(tc.tile_pool(name="io", bufs=3))
        h1_pool = ctx.enter_context(tc.tile_pool(name="h1", bufs=3))
        oh_pool = ctx.enter_context(tc.tile_pool(name="oh", bufs=4))
        st2 = ctx.enter_context(tc.tile_pool(name="st2", bufs=2))
        ps_h1 = ctx.enter_context(tc.tile_pool(name="ps_h1", bufs=2, space=PSUM))
        ps_s = ctx.enter_context(tc.tile_pool(name="ps_s", bufs=2, space=PSUM))
        ps_n = ctx.enter_context(tc.tile_pool(name="ps_n", bufs=1, space=PSUM))

        coff = 0
        for w in [wi for _ in range(repeat) for wi in range(NW)]:
            if w == 0:
                coff = 0
            Mw = int(M[w])
            wsl = slice(w * WIN, (w + 1) * WIN)
            ps_s_t = ps_s.tile([HID, WIN], F32, tag="ps", name="ps_s_t")
            for t0 in range(0, Mw, TILE_C):
                cn = min(TILE_C, Mw - t0)
                width = cn * WIN
                esl = slice((coff + t0) * WIN, (coff + t0 + cn) * WIN)
                xg_t = io_pool.tile([NODE_IN, TILE_C * WIN], F32, tag="xg",
                                    name="xg_t")
                nc.sync.dma_start(xg_t[:, :width], io["xgT"][:, esl])
                ea_t = io_pool.tile([EDGE_OUT, TILE_C * WIN], F32, tag="ea",
                                    name="ea_t")
                nc.gpsimd.dma_start(ea_t[:, :width], io["eaT"][:, esl])

                ph = ps_h1.tile([WIN, TILE_C * WIN], F32, tag="ph", name="ph")
                for c in range(cn):
                    sl = slice(c * WIN, (c + 1) * WIN)
                    nc.tensor.matmul(ph[:, sl], xg_t[:, sl], W1a_lo_s[:],
                                     start=True, stop=False)
                    nc.tensor.matmul(ph[:, sl], ea_t[:, sl], W1a_hi_s[:],
                                     start=False, stop=not has_b1a)
                    if has_b1a:
                        nc.tensor.matmul(ph[:, sl], ones_s[:], b1a_s[:],
                                         start=False, stop=True)
                h1_t = h1_pool.tile([WIN, TILE_C * WIN], F32, tag="h1",
                                    name="h1_t")
                nc.scalar.activation(h1_t[:, :width], ph[:, :width], AF.Relu)
                for c in range(cn):
                    ci = coff + t0 + c
                    sl = slice(c * WIN, (c + 1) * WIN)
                    oh = oh_pool.tile([WIN, WIN], F32, tag="oh", name="oh")
                    nc.vector.tensor_scalar(oh[:], iota_s[:],
                                            colT_s[:, ci:ci + 1],
                                            wT_s[:, ci:ci + 1],
                                            ALU.is_equal, ALU.mult)
                    nc.tensor.matmul(ps_s_t[:], h1_t[:, sl], oh[:],
                                     start=(t0 + c == 0),
                                     stop=(t0 + c == Mw - 1))
            s_sb = st2.tile([HID, WIN], F32, tag="s_sb", name="s_sb")
            nc.vector.tensor_copy(s_sb[:], ps_s_t[:])
            pa = ps_n.tile([HID, WIN], F32, tag="pa", name="pa")
            nc.tensor.matmul(pa[:], W1b_s[:], s_sb[:],
                             start=True, stop=not has_b1b)
            if has_b1b:
                nc.tensor.matmul(pa[:], b1b_s[:], mask_s[:, wsl],
                                 start=False, stop=True)
            agg_sb = st2.tile([HID, WIN], F32, tag="agg", name="agg_sb")
            nc.vector.tensor_copy(agg_sb[:], pa[:])

            xT_t = st2.tile([NODE_IN, WIN], F32, tag="xT", name="xT_t")
            nc.gpsimd.dma_start(xT_t[:], io["xT"][:, wsl])
            ub_t = st2.tile([GLOBAL_IN, WIN], F32, tag="ub", name="ub_t")
            nc.gpsimd.dma_start(ub_t[:], io["ubT"][:, wsl])
            pz = ps_n.tile([HID, WIN], F32, tag="pz", name="pz")
            nc.tensor.matmul(pz[:], W2a_x_s[:], xT_t[:], start=True, stop=False)
            nc.tensor.matmul(pz[:], W2a_agg_s[:], agg_sb[:],
                             start=False, stop=False)
            nc.tensor.matmul(pz[:], W2a_u_s[:], ub_t[:], start=False, stop=True)
            z1 = st2.tile([HID, WIN], F32, tag="z1", name="z1")
            nc.scalar.activation(z1[:], pz[:], AF.Relu, bias=b2a_s[:])
            po = ps_n.tile([NODE_OUT, WIN], F32, tag="po", name="po")
            nc.tensor.matmul(po[:], W2b_s[:], z1[:], start=True, stop=True)
            ot = st2.tile([NODE_OUT, WIN], F32, tag="ot", name="ot")
            nc.vector.tensor_scalar(ot[:], po[:], b2b_s[:], None, ALU.add)
            nc.sync.dma_start(io["outT"][:, wsl], ot[:])
            coff += Mw


def build(inputs, repeat=1):
    """Compile the SPMD kernel for the given inputs. Returns (nc, in_maps, dims)."""
    in_maps, M, dims = _plan(inputs)
    nc = bacc.Bacc(get_trn_type() or "TRN2", target_bir_lowering=False,
                   debug=False, num_devices=N_CORES)
    io = {}
    for name, arr in in_maps[0].items():
        io[name] = nc.dram_tensor(name, list(arr.shape), F32,
                                  kind="ExternalInput")
    io["outT"] = nc.dram_tensor("outT", [dims["NODE_OUT"], dims["NPAD"]], F32,
                                kind="ExternalOutput")
    with tile.TileContext(nc) as tc:
        _emit(tc, io, M, dims, repeat=repeat)
    nc.compile()
    return nc, in_maps, dims


def assemble(results, dims):
    N, NPC, NODE_OUT = dims["N"], dims["NPC"], dims["NODE_OUT"]
    out = np.empty((N, NODE_OUT), np.float32)
    for c in range(N_CORES):
        out[c * NPC:(c + 1) * NPC] = results[c]["outT"].T[:NPC]
    return out


def run(inputs, trace=False):
    nc, in_maps, dims = build(inputs)
    res = run_bass_kernel_spmd(nc, in_maps, list(range(N_CORES)), trace=trace)
    return assemble(res.results, dims), res


def kernel(**inputs):
    out, _ = run(inputs)
    return out
